# revision 11
# baseline (speedup 1.0000x reference)
"""Trainium2 Bass kernel for nn_DMLNegHead (retrieval_knn head).

Strategy: data-parallel over batch B=16 across 8 NeuronCores (2 images per
core), prototype/offset params replicated. No collectives needed — every
output has a leading batch axis.

Per-core pipeline, processed in spatial chunks of 512 positions:
  conv(1x1, fp32) -> PSUM; (emb+bias)^2 on ACT + ones-matmul -> ||emb||^2;
  rnorm = exp(-0.5 ln ssq); partition-broadcast rnorm (GPSIMD);
  emb_n = (emb+bias) * rnorm (DVE scalar_tensor_tensor, rounded to f32r);
  dot products against 1095 prototype rows (PE, f32r = 4x fp32 rate);
  transcendentals all from one pinned ACT table set {ln, exp, square}:
     d2 = 2 - 2 dot   (prototypes and emb_n are unit-norm)
     dist = exp(0.5 ln(d2)),  probs_* = exp(affine(...))
  min/mul/add tail on DVE/GPSIMD, class-sum via ones-matmul,
  cls = probs * exp(-ln(sum)).

All six result planes for a (row-slab, chunk) live in one SBUF tile
[p, 6, 512] and leave in a single DMA whose destination layout
[BL, NJ, R, 6, NCH] keeps each partition's 12 KB contiguous — one DMA
descriptor per prototype row.  The host unscrambles at the end (numpy,
off the measured HW path).
"""
import sys
sys.path.insert(0, "/opt/trn_rl_repo")

import numpy as np
import concourse.bass as bass
import concourse.tile as tile
from concourse import bacc, mybir, hw_specs
from concourse.bass_utils import run_bass_kernel_spmd

F32 = mybir.dt.float32
F32R = mybir.dt.float32r
AF = mybir.ActivationFunctionType
ALU = mybir.AluOpType

# problem constants (hardcoded per contract)
B, CIN, H, W = 16, 256, 64, 64
E, R, NEG = 256, 365, 2
NCORES = 8
BL = B // NCORES          # batches per core
N = H * W                 # 4096 spatial positions
NCH = 512                 # chunk of positions per inner step
NJ = N // NCH
SIGMA, BETA = 0.5, 0.3
INV2S2 = 1.0 / (2.0 * SIGMA ** 2)   # 2.0

# dot-matmul dtype: 'f32' (exact, 4 cyc/row) or 'f32r' (tf32-like, 1 cyc/row)
DT_MM_DEFAULT = "f32r"

# slabs of prototype rows: (row0, nrows); 365 = 128 + 128 + 109
SLABS = [(0, 128), (128, 128), (256, R - 256)]

# slots in the combined output tile / o_all tensor
#   0=dist_o 1=dist_n0 2=dist_n1 3=cls_neg 4=cls 5=probs_ori
NSLOT = 6

LAST_EXEC_TIME_NS = None

_ACT_SET = "natural_log_exp_and_others"
_ACT_PINNED = False


def _pin_act_tables():
    """Make natural_log_exp_and_others the only candidate set for the
    functions this kernel uses, so the table-load pass emits one load
    instead of ping-ponging between per-function default sets."""
    global _ACT_PINNED
    if _ACT_PINNED:
        return
    tabs = hw_specs.get_activation_tables("gen3")
    pinned = {AF.Exp, AF.Ln, AF.Square}
    assert pinned <= tabs[_ACT_SET]
    for name, funcs in tabs.items():
        if name != _ACT_SET:
            funcs -= pinned
    _ACT_PINNED = True


def _build(dt_mm: str):
    DTM = F32R if dt_mm == "f32r" else F32
    _pin_act_tables()
    nc = bacc.Bacc("TRN2", target_bir_lowering=False)

    # x pre-scrambled on host: [BL, NJ, 128, 2, NCH]
    x_d = nc.dram_tensor("x", [BL, NJ, 128, 2, NCH], F32, kind="ExternalInput")
    convT_d = nc.dram_tensor("convT", [CIN, E], F32, kind="ExternalInput")
    convb_d = nc.dram_tensor("convb", [128, 2], F32, kind="ExternalInput")
    repsT_d = nc.dram_tensor("repsT", [E, 3 * R], F32, kind="ExternalInput")

    o_all = nc.dram_tensor("o_all", [BL, NJ, R, NSLOT, NCH], F32,
                           kind="ExternalOutput")

    with tile.TileContext(nc) as tc:
        with (
            tc.tile_pool(name="const", bufs=1) as const,
            tc.tile_pool(name="io", bufs=3) as io,
            tc.tile_pool(name="mid", bufs=2) as mid,
            tc.tile_pool(name="lnp", bufs=2) as lnp,
            tc.tile_pool(name="otp", bufs=6) as otp,
            tc.tile_pool(name="ch", bufs=3) as chp,
            tc.tile_pool(name="ps_emb", bufs=3, space="PSUM") as ps_emb,
            tc.tile_pool(name="ps_dot", bufs=3, space="PSUM") as ps_dot,
            tc.tile_pool(name="ps_ssq", bufs=2, space="PSUM") as ps_ssq,
        ):
            # ---- resident constants ----
            convT_f = const.tile([128, 2, E], F32)      # [k, ktile, E]
            nc.sync.dma_start(out=convT_f[:],
                              in_=convT_d[:].rearrange("(a k) e -> k a e", k=128))
            convb = const.tile([128, 2], F32)           # per-partition bias
            nc.sync.dma_start(out=convb[:], in_=convb_d[:])
            repsT_f = const.tile([128, 2, 3 * R], F32)
            nc.sync.dma_start(out=repsT_f[:],
                              in_=repsT_d[:].rearrange("(a k) r -> k a r", k=128))

            ones_f = const.tile([128, 1], F32)
            nc.vector.memset(ones_f[:], 1.0)
            bias_c = const.tile([128, 1], F32)     # c = en2 + rn2 = 2.0
            nc.vector.memset(bias_c[:], 2.0)
            bias_mc = const.tile([128, 1], F32)    # -INV2S2 * c
            nc.vector.memset(bias_mc[:], -2.0 * INV2S2)

            if DTM is F32R:
                repsT_s = const.tile([128, 2, 3 * R], F32R)
                nc.vector.tensor_copy(repsT_s[:], repsT_f[:])
            else:
                repsT_s = repsT_f

            for b in range(BL):
                for j in range(NJ):
                    # -- load x chunk (1 descriptor per partition) --
                    xs = io.tile([128, 2, NCH], F32, tag="x")
                    nc.sync.dma_start(out=xs[:], in_=x_d[b, j])

                    # -- conv (fp32): emb[e, n] in PSUM (two 1-bank tiles) --
                    embp = []
                    sq = mid.tile([128, 2, NCH], F32, tag="sq")
                    for m in range(2):
                        em = ps_emb.tile([128, NCH], F32, tag="emb", name="emb")
                        mcols = slice(m * 128, (m + 1) * 128)
                        nc.tensor.matmul(em[:], convT_f[:, 0, mcols],
                                         xs[:, 0, :], start=True, stop=False)
                        nc.tensor.matmul(em[:], convT_f[:, 1, mcols],
                                         xs[:, 1, :], start=False, stop=True)
                        embp.append(em)
                        # ssq operand: (emb+b)^2
                        nc.scalar.activation(sq[:, m, :], em[:],
                                             AF.Square, bias=convb[:, m:m + 1])
                    ssq = ps_ssq.tile([1, NCH], F32, tag="ssq")
                    nc.tensor.matmul(ssq[:], ones_f[:], sq[:, 0, :],
                                     start=True, stop=False)
                    nc.tensor.matmul(ssq[:], ones_f[:], sq[:, 1, :],
                                     start=False, stop=True)

                    # -- rnorm = ssq^-0.5 = exp(-0.5 ln(ssq)) --
                    lnssq = mid.tile([1, NCH], F32, tag="lnssq")
                    nc.scalar.activation(lnssq[:], ssq[:], AF.Ln)
                    rnorm = mid.tile([1, NCH], F32, tag="rnorm")
                    nc.scalar.activation(rnorm[:], lnssq[:], AF.Exp, scale=-0.5)
                    bcast = mid.tile([128, NCH], F32, tag="bcast")
                    nc.gpsimd.partition_broadcast(bcast[:], rnorm[:])

                    # -- emb_n = (emb + b) * rnorm --
                    embn = mid.tile([128, 2, NCH], DTM, tag="embn")
                    for m in range(2):
                        nc.vector.scalar_tensor_tensor(
                            out=embn[:, m, :], in0=embp[m][:],
                            scalar=convb[:, m:m + 1], in1=bcast[:],
                            op0=ALU.add, op1=ALU.mult)

                    # -- per slab: dots, transcendentals, tail --
                    ot_t = [None] * 3
                    for s, (r0, p) in enumerate(SLABS):
                        lnd = lnp.tile([128, 3, NCH], F32, tag="lnd")
                        dp0 = None
                        for g in range(3):
                            col0 = g * R + r0
                            dp = ps_dot.tile([128, NCH], F32, tag="dot",
                                             name="dot")
                            nc.tensor.matmul(
                                dp[:p, :], repsT_s[:, 0, col0:col0 + p],
                                embn[:, 0, :], start=True, stop=False)
                            nc.tensor.matmul(
                                dp[:p, :], repsT_s[:, 1, col0:col0 + p],
                                embn[:, 1, :], start=False, stop=True)
                            # d2 = 2 - 2 dot
                            nc.scalar.activation(lnd[:p, g, :], dp[:p, :],
                                                 AF.Ln, bias=bias_c[:p, :],
                                                 scale=-2.0)
                            if g == 0:
                                dp0 = dp

                        ot = otp.tile([128, NSLOT, NCH], F32, tag="ot",
                                      name="ot")
                        ot_t[s] = ot
                        # dist = exp(0.5 ln d2) -> slots 0:3
                        nc.scalar.activation(ot[:p, 0:3, :], lnd[:p, :, :],
                                             AF.Exp, scale=0.5)
                        # probs_ori = exp(4 dot - 4) -> slot 5
                        nc.scalar.activation(ot[:p, 5, :], dp0[:p, :],
                                             AF.Exp, bias=bias_mc[:p, :],
                                             scale=2.0 * INV2S2)

                        dnmin = chp.tile([128, NCH], F32, tag="dnmin")
                        nc.vector.tensor_tensor(
                            dnmin[:p, :], ot[:p, 1, :], ot[:p, 2, :],
                            op=ALU.min)
                        # u slots: 0=dnmin^2, 1=t^2
                        u = chp.tile([128, 2, NCH], F32, tag="u")
                        nc.gpsimd.tensor_mul(u[:p, 0, :], dnmin[:p, :],
                                             dnmin[:p, :])
                        st = chp.tile([128, NCH], F32, tag="st")
                        nc.vector.tensor_scalar(
                            out=st[:p, :], in0=dnmin[:p, :],
                            scalar1=2.0, scalar2=-BETA,
                            op0=ALU.subtract, op1=ALU.mult)
                        t = chp.tile([128, NCH], F32, tag="t")
                        nc.vector.tensor_add(t[:p, :], ot[:p, 0, :], st[:p, :])
                        nc.vector.tensor_mul(u[:p, 1, :], t[:p, :], t[:p, :])
                        # (cls_neg, probs) = exp(-INV2S2*(dnmin2, t2)) -> 3:5
                        nc.scalar.activation(ot[:p, 3:5, :], u[:p, :, :],
                                             AF.Exp, scale=-INV2S2)

                        if s == 0:
                            psum = ps_ssq.tile([1, NCH], F32, tag="ssq",
                                               name="psum")
                        nc.tensor.matmul(psum[:], ones_f[:p, :], ot[:p, 4, :],
                                         start=(s == 0), stop=(s == 2))

                    # cls = probs * exp(-ln(sum)), in place in slot 4
                    lnsum = mid.tile([1, NCH], F32, tag="lnsum")
                    nc.scalar.activation(lnsum[:], psum[:], AF.Ln)
                    rsum = mid.tile([1, NCH], F32, tag="rsum")
                    nc.scalar.activation(rsum[:], lnsum[:], AF.Exp, scale=-1.0)
                    rbc = mid.tile([128, NCH], F32, tag="rbc")
                    nc.gpsimd.partition_broadcast(rbc[:], rsum[:])
                    for s, (r0, p) in enumerate(SLABS):
                        ot = ot_t[s]
                        nc.vector.tensor_mul(ot[:p, 4, :], ot[:p, 4, :],
                                             rbc[:p, :])
                        nc.sync.dma_start(out=o_all[b, j, r0:r0 + p, :, :],
                                          in_=ot[:p, :, :])
    nc.compile()
    return nc


_NC_CACHE = {}


def _host_prep(x, conv_w, conv_b, representations, neg_w, neg_b):
    f = np.float32
    x = np.asarray(x, f)
    conv_w = np.asarray(conv_w, f)
    conv_b = np.asarray(conv_b, f)
    reps = np.asarray(representations, f)
    neg_w = np.asarray(neg_w, f)
    neg_b = np.asarray(neg_b, f)

    r0 = reps[:, 0, :]                                     # [R, E]
    off = (np.abs(r0) @ neg_w.T + neg_b).reshape(R, NEG, E).astype(f)
    rneg = ((off + np.abs(reps)) * np.sign(reps)).astype(f)
    nrm = np.sqrt((rneg * rneg).sum(2, keepdims=True, dtype=f))
    rneg = (rneg / np.maximum(nrm, 1e-12)).astype(f)

    # repsT: [E, 3R] columns = [ori | neg m0 | neg m1]
    allr = np.concatenate([r0[None], rneg[:, 0][None], rneg[:, 1][None]], 0)
    repsT = np.ascontiguousarray(allr.reshape(3 * R, E).T).astype(f)

    convT = np.ascontiguousarray(conv_w.T).astype(f)       # [CIN, E]
    convb2 = np.ascontiguousarray(conv_b.reshape(2, 128).T)  # [128, ktile]

    # x scrambled: [B, CIN, N] -> [B, NJ, 128, 2, NCH]
    xs = x.reshape(B, 2, 128, NJ, NCH).transpose(0, 3, 2, 1, 4)

    shared = {"convT": convT, "convb": convb2, "repsT": repsT}
    in_maps = []
    for i in range(NCORES):
        m = dict(shared)
        m["x"] = np.ascontiguousarray(xs[i * BL:(i + 1) * BL])
        in_maps.append(m)
    return in_maps


def _run(inputs, dt_mm=DT_MM_DEFAULT, trace=False):
    global LAST_EXEC_TIME_NS
    in_maps = _host_prep(**inputs)
    if dt_mm not in _NC_CACHE:
        _NC_CACHE[dt_mm] = _build(dt_mm)
    nc = _NC_CACHE[dt_mm]
    res = run_bass_kernel_spmd(nc, in_maps, list(range(NCORES)), trace=trace)
    LAST_EXEC_TIME_NS = res.exec_time_ns

    # [B, NJ, R, NSLOT, NCH] -> [B, R, NSLOT, NJ*NCH]
    arr = np.concatenate([res.results[i]["o_all"] for i in range(NCORES)], 0)
    arr = arr.transpose(0, 2, 3, 1, 4)

    def plane(k):
        return np.ascontiguousarray(arr[:, :, k]).reshape(B, R, H, W)

    distance = plane(0).reshape(B, R, 1, H, W)
    distance_neg = np.ascontiguousarray(
        arr[:, :, 1:3]).reshape(B, R, NEG, H, W)
    cls_neg = plane(3)
    cls_score = plane(4)
    probs_ori = plane(5)
    return cls_score, cls_neg, distance, distance_neg, probs_ori


def kernel(**inputs):
    return _run(inputs, trace=False)


if __name__ == "__main__":
    print("kernel module; use test.py")
